# revision 13
# baseline (speedup 1.0000x reference)
"""Trainium2 Bass kernel for nn_Attention_Conv_surface (gnn_message_passing).

Math (per batch b):
  neighbors = vertices[idx]                          # (V, N, 3)
  dirn = normalize(neighbors - vertices[:, None])    # (V, N, 3)
  theta_d = sum_s max_n relu(dirn @ sdn_d)           # (V, K) for d in {q,k,v}
  qkv = theta @ W.T + b ; MHA over full VxV ; out = attn_out @ Wo.T + bo

Device strategy (v2):
  * theta matmuls use 4x row-tiling (tile_position): 4 neighbors run
    concurrently on 32-row PE tiles.  Per neighbor the contraction is 6 rows:
    (dirh; dirl) against (sdh; sdh), i.e. sdh * dirn_fp32 exactly; only the
    support-direction rounding (~2^-9, fixed per sk column) remains.
  * t4 operand layout [128 rows, grp, 512v] is built with DMA-xbar transposes
    (no PE/ACT involvement); rows 6..31 of each 32-row slot are garbage and
    are killed by zero rows in the dense per-chunk lhsT.
  * max over n: 4-neighbor PSUM supertiles [128, 2048]; most groups take the
    ACT route (fused relu + bf16 cast to SBUF, then bf16 2x tensor_tensor max
    on DVE), the rest are maxed straight out of PSUM on DVE.  The ACT/DVE
    ratio (ACT_GRPS) balances the two engines.
  * attention: scores transposed with augmented operands qa=[qh/4;-m], ka=[kh;1]
    in an x3 block layout; m is a per-head norm bound (4*max|qf|*max|kf|),
    valid since softmax is shift-invariant and exp(s-m) stays in [e^-2m, 1].
  * v head is produced directly transposed: va[v,dk] via matmuls with
    lhsT=theta_v columns, accumulating the Wv hi/lo product; bv is folded into
    the output bias on the host (exact, since attention rows sum to 1).
  * exp on ACT in [128,1024] two-bank batches; PV augments v with a ones row
    so the softmax denominator falls out of the same matmul.

Sharding: 8 cores = (batch 0..3) x (query half 0..1).  Each core computes
k/v thetas for the full batch (duplicated within the pair) and q theta +
attention for its own 1024 queries.  Identical SPMD program; the query half is
selected by feeding each core a half-rolled permutation of its batch's data.
"""

import numpy as np

BS, V, N, S, K, H = 4, 2048, 32, 4, 64, 4
DK = K // H
VQ = V // 2          # queries per core
NCH = 6              # sk chunks of 128 (768 total = 3 dirs * 256)
EPS = 1e-12
AUG = 81             # rows used of the x3-block score operands
ACT_GRPS = 7         # of 8 neighbor-groups per (ch,g): routed via ACT relu-copy

_CACHE = {}


def _build_program():
    import concourse.bass as bass
    import concourse.mybir as mybir
    import concourse.tile as tile
    from concourse import bacc
    from contextlib import ExitStack

    f32 = mybir.dt.float32
    bf16 = mybir.dt.bfloat16
    Alu = mybir.AluOpType
    Act = mybir.ActivationFunctionType

    nc = bacc.Bacc("TRN2", target_bir_lowering=False, debug=False)

    # ---- DRAM I/O ----
    verts_d = nc.dram_tensor("verts", [V, 3], f32, kind="ExternalInput").ap()
    gath_d = nc.dram_tensor("gath", [V, N, 3], f32, kind="ExternalInput").ap()
    sdt_d = nc.dram_tensor("sdt", [NCH, 128, 128], bf16, kind="ExternalInput").ap()
    ident_d = nc.dram_tensor("ident", [128, 128], f32, kind="ExternalInput").ap()
    identb_d = nc.dram_tensor("identb", [128, 128], bf16, kind="ExternalInput").ap()
    wst_d = nc.dram_tensor("wst", [4, 128, K], bf16, kind="ExternalInput").ap()
    wl_d = nc.dram_tensor("wl", [4, K, K], bf16, kind="ExternalInput").ap()
    bh_d = nc.dram_tensor("bh", [DK, 16], f32, kind="ExternalInput").ap()
    bo_d = nc.dram_tensor("bo_col", [K, 1], f32, kind="ExternalInput").ap()
    ones_row_d = nc.dram_tensor("ones_row", [1, V], bf16, kind="ExternalInput").ap()
    ones_col_d = nc.dram_tensor("ones_col", [128, V // 128], bf16, kind="ExternalInput").ap()
    out_d = nc.dram_tensor("out_t", [K, VQ], f32, kind="ExternalOutput").ap()

    NVT = V // 128  # 16 vertex tiles

    with tile.TileContext(nc) as tc:
        with (
            tc.tile_pool(name="const", bufs=1) as cpool,
        ):
            # ---- persistent constants ----
            ident = cpool.tile([128, 128], f32)
            nc.sync.dma_start(ident[:], ident_d[:])
            identb = cpool.tile([128, 128], bf16)
            nc.sync.dma_start(identb[:], identb_d[:])
            sdt = cpool.tile([128, NCH, 128], bf16)
            nc.sync.dma_start(sdt[:], sdt_d.rearrange("c p m -> p c m"))
            wst = cpool.tile([128, 4, K], bf16)
            nc.sync.dma_start(wst[:], wst_d.rearrange("w a b -> a w b"))
            wl = cpool.tile([K, 4, K], bf16)
            nc.sync.dma_start(wl[:], wl_d.rearrange("w a b -> a w b"))
            bh = cpool.tile([DK, 16], f32)
            nc.sync.dma_start(bh[:], bh_d[:])
            bo = cpool.tile([K, 1], f32)
            nc.sync.dma_start(bo[:], bo_d[:])
            # persistent theta^T splits [h-rows 0:64 | l-rows 64:128]
            th_q = cpool.tile([128, VQ], bf16)
            th_k = cpool.tile([128, V], bf16)
            th_v = cpool.tile([128, V], bf16)
            # score operand tiles (x3 block layout), zeroed once; double-buffered
            # by head parity so head h+1's builds overlap head h's attention
            qa3_t, ka3_t, va_t = [], [], []
            for hb in range(2):
                qa3 = cpool.tile([96, VQ], bf16, name=f"qa3_{hb}")
                nc.vector.memset(qa3[:], 0.0)
                qa3_t.append(qa3)
                ka3 = cpool.tile([96, V], bf16, name=f"ka3_{hb}")
                nc.vector.memset(ka3[:], 0.0)
                nc.sync.dma_start(ka3[DK : DK + 1, :], ones_row_d[:])
                nc.sync.dma_start(ka3[64 + DK : 64 + DK + 1, :], ones_row_d[:])
                ka3_t.append(ka3)
                va = cpool.tile([128, V // 128, DK + 1], bf16, name=f"va_{hb}")
                nc.sync.dma_start(
                    va[:, :, DK : DK + 1].rearrange("p a b -> p (a b)"), ones_col_d[:]
                )
                va_t.append(va)
            O = cpool.tile([128, 8, K], f32)       # [128q, qt, 64]
            OT2 = cpool.tile([128, VQ], bf16)      # [OTh | OTl]
            outsb = cpool.tile([K, VQ], f32)
            # theta accumulation staging (reused per pr)
            xf = cpool.tile([K, V], f32)
            xcA = cpool.tile([K, V], f32)
            xcB = cpool.tile([K, V], f32)

            theta_stack = ExitStack()
            vtpool = theta_stack.enter_context(tc.tile_pool(name="vt", bufs=3))
            dxpool = theta_stack.enter_context(tc.tile_pool(name="dx", bufs=2))
            t4pool = theta_stack.enter_context(tc.tile_pool(name="t4p", bufs=1))
            pspool = theta_stack.enter_context(
                tc.tile_pool(name="ps", bufs=2, space="PSUM")
            )
            stpool = theta_stack.enter_context(tc.tile_pool(name="st", bufs=2))
            accpool = theta_stack.enter_context(tc.tile_pool(name="acc", bufs=2))

            t4s = []
            for g in range(4):
                t4_t = t4pool.tile([128, 8, 512], bf16, tag=f"t4_{g}", name=f"t4_{g}")
                t4s.append(t4_t)

            # ---- phase 1: per-vtile edge math -> dx staging -> DMA transposes ----
            for vt in range(NVT):
                g, vt4 = vt // 4, vt % 4
                vsl = slice(vt * 128, vt * 128 + 128)
                gath = vtpool.tile([128, N, 3], f32, tag="gath")
                nc.sync.dma_start(gath[:], gath_d[vsl, :, :])
                cent = vtpool.tile([128, 3], f32, tag="cent")
                nc.sync.dma_start(cent[:], verts_d[vsl, :])
                diff = vtpool.tile([128, N, 3], f32, tag="diff")
                for c in range(3):
                    nc.vector.tensor_tensor(
                        out=diff[:, :, c],
                        in0=gath[:, :, c],
                        in1=cent[:, c : c + 1].to_broadcast([128, N]),
                        op=Alu.subtract,
                    )
                dsq = vtpool.tile([128, N, 3], f32, tag="dsq")
                nc.scalar.square(dsq[:], diff[:])
                nsq = vtpool.tile([128, N], f32, tag="nsq")
                nc.vector.reduce_sum(nsq[:], dsq[:], axis=mybir.AxisListType.X)
                nrm = vtpool.tile([128, N], f32, tag="nrm")
                nc.scalar.sqrt(nrm[:], nsq[:])
                nc.vector.tensor_scalar_max(nrm[:], nrm[:], EPS)
                invn = vtpool.tile([128, N], f32, tag="invn")
                nc.vector.reciprocal(invn[:], nrm[:])
                tdn = vtpool.tile([128, N, 3], f32, tag="tdn")
                nc.vector.tensor_tensor(
                    out=tdn[:],
                    in0=diff[:],
                    in1=invn[:].to_broadcast([128, N, 3]),
                    op=Alu.mult,
                )
                # dx staging: [128, grp(8), slot(4), 32rows]; rows 0:3 = dirh,
                # rows 3:6 = dirl (neighbor n = 4*grp + slot)
                dx = dxpool.tile([128, 8, 4, 32], bf16, tag="dx")
                if vt < 2:
                    # zero the two rotating staging buffers once: garbage rows
                    # hit zero lhsT rows, and 0*NaN would poison the PSUM
                    nc.vector.memset(dx[:], 0.0)
                tdn_r = tdn[:].rearrange("p (g j) c -> p g j c", g=8)
                nc.vector.tensor_copy(dx[:, :, :, 0:3], tdn_r)
                nc.vector.tensor_tensor(
                    out=dx[:, :, :, 3:6], in0=tdn_r, in1=dx[:, :, :, 0:3],
                    op=Alu.subtract,
                )
                # batched xbar transpose: out[r, g8, v] = dx[v, 128*g8 + r]
                nc.sync.dma_start_transpose(
                    t4s[g][:, :, vt4 * 128 : vt4 * 128 + 128],
                    dx[:].rearrange("p a b c -> p (a b c)"),
                )

            # ---- phase 2: theta matmuls (4x row-tiled); relu+max; s-sum ----
            for pr in (1, 2, 0):  # k, v, q  (q last; th_q only needs own half)
                ngr = 2 if pr == 0 else 4
                for ch in range(2):
                    lhs = sdt[:, 2 * pr + ch, :]
                    for g in range(ngr):
                        acc = accpool.tile([128, 512], bf16, tag="acc")
                        for grp in range(8):
                            ps = pspool.tile([128, 2048], f32, tag="big")
                            for j in range(4):
                                nc.tensor.matmul(
                                    out=ps[:, 512 * j : 512 * j + 512],
                                    lhsT=lhs[32 * j : 32 * j + 32, :],
                                    rhs=t4s[g][32 * j : 32 * j + 32, grp, :],
                                    start=True,
                                    stop=True,
                                    tile_position=(32 * j, 0),
                                )
                            if grp < ACT_GRPS:
                                st = stpool.tile([128, 2048], bf16, tag="st")
                                nc.scalar.activation(st[:], ps[:], Act.Relu)
                                tmp = stpool.tile([128, 1024], bf16, tag="tmp")
                                nc.vector.tensor_tensor(
                                    out=tmp[:], in0=st[:, 0:1024],
                                    in1=st[:, 1024:2048], op=Alu.max,
                                )
                                if grp == 0:
                                    nc.vector.tensor_tensor(
                                        out=acc[:], in0=tmp[:, 0:512],
                                        in1=tmp[:, 512:1024], op=Alu.max,
                                    )
                                else:
                                    t2 = stpool.tile([128, 512], bf16, tag="t2")
                                    nc.vector.tensor_tensor(
                                        out=t2[:], in0=tmp[:, 0:512],
                                        in1=tmp[:, 512:1024], op=Alu.max,
                                    )
                                    nc.vector.tensor_tensor(
                                        out=acc[:], in0=t2[:], in1=acc[:],
                                        op=Alu.max,
                                    )
                            else:
                                for j in range(4):
                                    nc.vector.tensor_tensor(
                                        out=acc[:],
                                        in0=ps[:, 512 * j : 512 * j + 512],
                                        in1=acc[:],
                                        op=Alu.max,
                                    )
                        if ACT_GRPS < 8:
                            nc.vector.tensor_scalar_max(acc[:], acc[:], 0.0)
                        gsl = slice(g * 512, g * 512 + 512)
                        # DVE TT needs both SBUF inputs at the same base
                        # partition; shift the upper s-half down via DMA.
                        shp = accpool.tile([K, 512], bf16, tag="shp")
                        nc.sync.dma_start(shp[:], acc[K:128, :])
                        xc = xcA if ch == 0 else xcB
                        nc.vector.tensor_tensor(
                            out=xc[:, gsl], in0=acc[0:K, :], in1=shp[:],
                            op=Alu.add,
                        )
                        if ch == 1:
                            nc.vector.tensor_tensor(
                                out=xf[:, gsl], in0=xcA[:, gsl], in1=xcB[:, gsl],
                                op=Alu.add,
                            )
                # theta hi/lo split [128, VV]: rows 0:64 hi, 64:128 lo
                th = {0: th_q, 1: th_k, 2: th_v}[pr]
                vv = VQ if pr == 0 else V
                nc.vector.tensor_copy(th[0:K, :], xf[:, 0:vv])
                nc.vector.tensor_tensor(
                    out=th[K:128, :], in0=xf[:, 0:vv], in1=th[0:K, :],
                    op=Alu.subtract,
                )
            theta_stack.close()

            # ---- phase 3+4: per-head projection + attention ----
            attn_stack = ExitStack()
            atpool = attn_stack.enter_context(tc.tile_pool(name="attn", bufs=2))
            epool = attn_stack.enter_context(tc.tile_pool(name="epool", bufs=3))
            psxpool = attn_stack.enter_context(
                tc.tile_pool(name="psx", bufs=2, space="PSUM")
            )
            pstpool = attn_stack.enter_context(
                tc.tile_pool(name="pst", bufs=2, space="PSUM")
            )
            stppool = attn_stack.enter_context(
                tc.tile_pool(name="stp", bufs=2, space="PSUM")
            )

            for h in range(H):
                hsl = slice(DK * h, DK * h + DK)
                qa3, ka3, va = qa3_t[h % 2], ka3_t[h % 2], va_t[h % 2]
                # q/k projections for this head: 2-matmul hi/lo scheme
                heads = {}
                for wi, (th, vv, nm) in enumerate(
                    ((th_q, VQ, "qf"), (th_k, V, "kf"))
                ):
                    hf = atpool.tile([DK, vv], f32, tag=nm)
                    heads[nm] = hf
                    for tt in range(vv // 512):
                        sl = slice(tt * 512, tt * 512 + 512)
                        pp = psxpool.tile([DK, 512], f32, tag="xps")
                        nc.tensor.matmul(
                            out=pp[:], lhsT=wst[:, wi, hsl], rhs=th[:, sl],
                            start=True, stop=False,
                        )
                        nc.tensor.matmul(
                            out=pp[:], lhsT=wl[:, wi, hsl], rhs=th[0:K, sl],
                            start=False, stop=True,
                        )
                        nc.scalar.activation(
                            hf[:, sl], pp[:], Act.Identity,
                            bias=bh[:, wi * 4 + h : wi * 4 + h + 1],
                        )
                qf, kf = heads["qf"], heads["kf"]

                # v head, directly transposed: va[v, dk] accumulating hi/lo
                for hv in range(2):
                    psv = pstpool.tile([128, 128], f32, tag="small")
                    for c8 in range(8):
                        c = hv * 8 + c8
                        csl = slice(c * 128, c * 128 + 128)
                        osl = slice(16 * c8, 16 * c8 + 16)
                        nc.tensor.matmul(
                            out=psv[:, osl], lhsT=th_v[:, csl],
                            rhs=wst[:, 2, hsl],
                            start=(c8 == 0), stop=False,
                        )
                        nc.tensor.matmul(
                            out=psv[:, osl], lhsT=th_v[0:K, csl],
                            rhs=wl[:, 2, hsl],
                            start=False, stop=(c8 == 7),
                        )
                    nc.vector.tensor_copy(
                        va[:, hv * 8 : hv * 8 + 8, 0:DK],
                        psv[:].rearrange("p (a b) -> p a b", a=8),
                    )

                # ka3 blocks: [0:16]=kah, [32:48]=kal, [64:80]=kah
                nc.vector.tensor_copy(ka3[0:DK, :], kf[:])
                nc.vector.tensor_tensor(
                    out=ka3[32 : 32 + DK, :], in0=kf[:], in1=ka3[0:DK, :],
                    op=Alu.subtract,
                )
                nc.vector.tensor_copy(ka3[64 : 64 + DK, :], ka3[0:DK, :])
                # qa3 blocks: [0:16]=qah, [32:48]=qah, [64:80]=qal (q/4)
                q4 = atpool.tile([DK, VQ], f32, tag="q4")
                nc.scalar.mul(q4[:], qf[:], 0.25)
                nc.vector.tensor_copy(qa3[0:DK, :], q4[:])
                nc.vector.tensor_copy(qa3[32 : 32 + DK, :], qa3[0:DK, :])
                nc.vector.tensor_tensor(
                    out=qa3[64 : 64 + DK, :], in0=q4[:], in1=qa3[0:DK, :],
                    op=Alu.subtract,
                )
                # norm-bound shift: m = 4 * max|qf| * max|kf| (>= max score);
                # softmax is shift-invariant, exp(s-m) in [e^-2m, 1], m ~ 1.
                qm = atpool.tile([DK, 1], f32, tag="qm")
                nc.vector.reduce_max(
                    qm[:], qf[:], axis=mybir.AxisListType.X,
                    apply_absolute_value=True,
                )
                km = atpool.tile([DK, 1], f32, tag="km")
                nc.vector.reduce_max(
                    km[:], kf[:], axis=mybir.AxisListType.X,
                    apply_absolute_value=True,
                )
                qmr = atpool.tile([1, DK], f32, tag="qmr")
                nc.sync.dma_start(qmr[:], qm[:].rearrange("p a -> a p"))
                kmr = atpool.tile([1, DK], f32, tag="kmr")
                nc.sync.dma_start(kmr[:], km[:].rearrange("p a -> a p"))
                qs1 = atpool.tile([1, 1], f32, tag="qs1")
                nc.vector.reduce_max(qs1[:], qmr[:], axis=mybir.AxisListType.X)
                ks1 = atpool.tile([1, 1], f32, tag="ks1")
                nc.vector.reduce_max(ks1[:], kmr[:], axis=mybir.AxisListType.X)
                ms = atpool.tile([1, 1], f32, tag="ms")
                nc.vector.tensor_tensor(
                    out=ms[:], in0=qs1[:], in1=ks1[:], op=Alu.mult
                )
                nc.vector.tensor_scalar_mul(ms[:], ms[:], -4.0)
                # DVE can't write a single partition at offset 16; stage the
                # broadcast row at base 0 and DMA it into place.
                mrow = atpool.tile([1, VQ], bf16, tag="mrow")
                nc.vector.tensor_copy(mrow[:], ms[:].to_broadcast([1, VQ]))
                nc.sync.dma_start(qa3[DK : DK + 1, :], mrow[:])

                # ST' + exp + PV, software-pipelined: each ST' pair is issued
                # one step ahead of the matching PV pair so the in-order PE
                # queue never stalls behind a PV that waits on its exp.
                NK2 = V // 256
                for qs in range(VQ // 512):
                    pv = psxpool.tile([DK + 1, 512], f32, tag="xps")

                    def emit_st(k2):
                        stp = stppool.tile([128, 1024], f32, tag="stp", name="stp")
                        for kk in range(2):
                            kt = k2 * 2 + kk
                            nc.tensor.matmul(
                                out=stp[:, 512 * kk : 512 * kk + 512],
                                lhsT=ka3[0:AUG, kt * 128 : kt * 128 + 128],
                                rhs=qa3[0:AUG, qs * 512 : qs * 512 + 512],
                                start=True,
                                stop=True,
                            )
                        return stp

                    stp_cur = emit_st(0)
                    for k2 in range(NK2):
                        e = epool.tile([128, 1024], bf16, tag="e")
                        nc.scalar.activation(e[:], stp_cur[:], Act.Exp)
                        if k2 + 1 < NK2:
                            stp_cur = emit_st(k2 + 1)
                        for kk in range(2):
                            kt = k2 * 2 + kk
                            nc.tensor.matmul(
                                out=pv[:],
                                lhsT=va[:, kt, :],
                                rhs=e[:, 512 * kk : 512 * kk + 512],
                                start=(kt == 0),
                                stop=(kt == V // 128 - 1),
                            )
                    pvs = atpool.tile([DK + 1, 512], f32, tag="pvs")
                    nc.vector.tensor_copy(pvs[:], pv[:])
                    for q4i in range(4):
                        qt = qs * 4 + q4i
                        pq = pstpool.tile([128, DK + 1], f32, tag="small")
                        nc.tensor.transpose(
                            pq[:], pvs[:, q4i * 128 : q4i * 128 + 128],
                            ident[0 : DK + 1, 0 : DK + 1],
                        )
                        rz = atpool.tile([128, 1], f32, tag="rz")
                        nc.vector.reciprocal(rz[:], pq[:, DK : DK + 1])
                        nc.vector.tensor_scalar_mul(O[:, qt, hsl], pq[:, 0:DK], rz[:])

            # ---- phase 5: O hi/lo transpose + final projection ----
            for qt in range(8):
                qsl = slice(qt * 128, qt * 128 + 128)
                oh = atpool.tile([128, K], bf16, tag="oh")
                nc.vector.tensor_copy(oh[:], O[:, qt, :])
                ol = atpool.tile([128, K], bf16, tag="ol")
                nc.vector.tensor_tensor(
                    out=ol[:], in0=O[:, qt, :], in1=oh[:], op=Alu.subtract
                )
                oph = pstpool.tile([K, 128], bf16, tag="small")
                nc.tensor.transpose(oph[:], oh[:], identb[:])
                nc.scalar.copy(OT2[0:K, qsl], oph[:])
                opl = pstpool.tile([K, 128], bf16, tag="small")
                nc.tensor.transpose(opl[:], ol[:], identb[:])
                nc.scalar.copy(OT2[K:128, qsl], opl[:])
            for qs in range(VQ // 512):
                sl = slice(qs * 512, qs * 512 + 512)
                fp = psxpool.tile([K, 512], f32, tag="xps")
                nc.tensor.matmul(
                    out=fp[:], lhsT=wst[:, 3, :], rhs=OT2[:, sl],
                    start=True, stop=False,
                )
                nc.tensor.matmul(
                    out=fp[:], lhsT=wl[:, 3, :], rhs=OT2[0:K, sl],
                    start=False, stop=True,
                )
                nc.scalar.activation(outsb[:, sl], fp[:], Act.Identity, bias=bo[:])
            nc.sync.dma_start(out_d[:], outsb[:])
            attn_stack.close()

    nc.compile()
    return nc


def _host_prep(inputs):
    """Build the 8 per-core input maps from full inputs."""
    import ml_dtypes

    bfd = ml_dtypes.bfloat16
    verts = np.ascontiguousarray(np.asarray(inputs["vertices"], dtype=np.float32))
    idx = np.ascontiguousarray(np.asarray(inputs["neighbor_index"]).astype(np.int32))

    sd = np.concatenate(
        [np.asarray(inputs["q_dirs"]), np.asarray(inputs["k_dirs"]),
         np.asarray(inputs["v_dirs"])], axis=1
    ).astype(np.float32)  # [3, 768]
    nrm = np.sqrt((sd * sd).sum(0, dtype=np.float32), dtype=np.float32)
    sdn = (sd / np.maximum(nrm, np.float32(EPS))).astype(np.float32)
    sdh = sdn.astype(bfd)

    # dense lhsT bank: [ch, 128, 128]; per 32-row slot j: rows 0:3 and 3:6
    # both carry sdh for the chunk's 128 sk columns (pairs with dirh; dirl)
    sdt = np.zeros((NCH, 128, 128), bfd)
    for ch in range(NCH):
        blk = sdh[:, ch * 128 : ch * 128 + 128]
        for j in range(4):
            sdt[ch, 32 * j + 0 : 32 * j + 3, :] = blk
            sdt[ch, 32 * j + 3 : 32 * j + 6, :] = blk

    # weights: wst [4, 128, 64] = [Wh.T ; Wh.T], wl [4, 64, 64] = Wl.T
    wst = np.zeros((4, 128, K), bfd)
    wlo = np.zeros((4, K, K), bfd)
    for wi, kk in enumerate(("Wq", "Wk", "Wv", "Wo")):
        wt_ = np.asarray(inputs[kk], dtype=np.float32).T
        wh_ = wt_.astype(bfd)
        wst[wi, 0:K, :] = wh_
        wst[wi, K:128, :] = wh_
        wlo[wi] = (wt_ - wh_.astype(np.float32)).astype(bfd)

    bh = np.zeros((DK, 16), np.float32)
    for wi, kk in enumerate(("bq", "bk", "bv", "bo")):
        bb_ = np.asarray(inputs[kk], dtype=np.float32)
        for h in range(H):
            bh[:, wi * 4 + h] = bb_[DK * h : DK * h + DK]
    # bv folded into output bias: attention rows sum to 1, so the +bv inside
    # vh passes through to x additively; x@Wo.T + bo == pv@Wo.T + (Wo@bv + bo)
    bo2 = (
        np.asarray(inputs["bo"], dtype=np.float32)
        + np.asarray(inputs["Wo"], dtype=np.float32)
        @ np.asarray(inputs["bv"], dtype=np.float32)
    ).reshape(K, 1)

    common = {
        "sdt": sdt,
        "ident": np.eye(128, dtype=np.float32),
        "identb": np.eye(128, dtype=np.float32).astype(bfd),
        "wst": wst,
        "wl": wlo,
        "bh": bh,
        "bo_col": bo2,
        "ones_row": np.ones((1, V), bfd),
        "ones_col": np.ones((128, V // 128), bfd),
    }

    in_maps = []
    for core in range(8):
        bb, half = core // 2, core % 2
        if half == 0:
            vb, ib = verts[bb], idx[bb]
        else:
            perm = np.concatenate([np.arange(VQ, V), np.arange(0, VQ)])
            vb = verts[bb][perm]
            ib = np.where(idx[bb][perm] >= VQ, idx[bb][perm] - VQ, idx[bb][perm] + VQ)
        in_maps.append({
            "verts": np.ascontiguousarray(vb),
            "gath": np.ascontiguousarray(vb[ib]),
            **common,
        })
    return in_maps


def run(inputs, trace=False, trace_kwargs=None):
    from concourse.bass_utils import run_bass_kernel_spmd

    if "nc" not in _CACHE:
        _CACHE["nc"] = _build_program()
    nc = _CACHE["nc"]
    in_maps = _host_prep(inputs)
    res = run_bass_kernel_spmd(
        nc, in_maps, core_ids=list(range(8)), trace=trace,
        **(trace_kwargs or {}),
    )
    out = np.zeros((BS, V, K), np.float32)
    for core in range(8):
        bb, half = core // 2, core % 2
        ot = res.results[core]["out_t"]  # [64, 1024]
        out[bb, half * VQ : half * VQ + VQ, :] = ot.T
    return out, res


def kernel(**inputs) -> np.ndarray:
    out, _ = run(inputs, trace=False)
    return out


# revision 15
# speedup vs baseline: 1.1235x; 1.1235x over previous
"""Trainium2 Bass kernel for nn_Attention_Conv_surface (gnn_message_passing).

Math (per batch b):
  neighbors = vertices[idx]                          # (V, N, 3)
  dirn = normalize(neighbors - vertices[:, None])    # (V, N, 3)
  theta_d = sum_s max_n relu(dirn @ sdn_d)           # (V, K) for d in {q,k,v}
  qkv = theta @ W.T + b ; MHA over full VxV ; out = attn_out @ Wo.T + bo

Device strategy (v2):
  * theta matmuls use 4x row-tiling (tile_position): 4 neighbors run
    concurrently on 32-row PE tiles.  Per neighbor the contraction is 6 rows:
    (dirh; dirl) against (sdh; sdh), i.e. sdh * dirn_fp32 exactly; only the
    support-direction rounding (~2^-9, fixed per sk column) remains.
  * t4 operand layout [128 rows, grp, 512v] is built with DMA-xbar transposes
    (no PE/ACT involvement); rows 6..31 of each 32-row slot are garbage and
    are killed by zero rows in the dense per-chunk lhsT.
  * max over n: 4-neighbor PSUM supertiles [128, 2048]; most groups take the
    ACT route (fused relu + bf16 cast to SBUF, then bf16 2x tensor_tensor max
    on DVE), the rest are maxed straight out of PSUM on DVE.  The ACT/DVE
    ratio (ACT_GRPS) balances the two engines.
  * attention: scores transposed with augmented operands qa=[qh/4;-m], ka=[kh;1]
    in an x3 block layout; m is a per-head norm bound (4*max|qf|*max|kf|),
    valid since softmax is shift-invariant and exp(s-m) stays in [e^-2m, 1].
  * v head is produced directly transposed: va[v,dk] via matmuls with
    lhsT=theta_v columns, accumulating the Wv hi/lo product; bv is folded into
    the output bias on the host (exact, since attention rows sum to 1).
  * exp on ACT in [128,1024] two-bank batches; PV augments v with a ones row
    so the softmax denominator falls out of the same matmul.

Sharding: 8 cores = (batch 0..3) x (query half 0..1).  Each core computes
k/v thetas for the full batch (duplicated within the pair) and q theta +
attention for its own 1024 queries.  Identical SPMD program; the query half is
selected by feeding each core a half-rolled permutation of its batch's data.
"""

import numpy as np

BS, V, N, S, K, H = 4, 2048, 32, 4, 64, 4
DK = K // H
VQ = V // 2          # queries per core
NCH = 6              # sk chunks of 128 (768 total = 3 dirs * 256)
EPS = 1e-12
AUG = 81             # rows used of the x3-block score operands
ACT_GRPS = 7         # of 8 neighbor-groups per (ch,g): routed via ACT relu-copy

_CACHE = {}


def _build_program():
    import concourse.bass as bass
    import concourse.mybir as mybir
    import concourse.tile as tile
    from concourse import bacc
    from contextlib import ExitStack

    f32 = mybir.dt.float32
    bf16 = mybir.dt.bfloat16
    Alu = mybir.AluOpType
    Act = mybir.ActivationFunctionType

    nc = bacc.Bacc("TRN2", target_bir_lowering=False, debug=False)

    # ---- DRAM I/O ----
    verts_d = nc.dram_tensor("verts", [V, 3], f32, kind="ExternalInput").ap()
    gath_d = nc.dram_tensor("gath", [V, N, 3], f32, kind="ExternalInput").ap()
    sdt_d = nc.dram_tensor("sdt", [NCH, 128, 128], bf16, kind="ExternalInput").ap()
    ident_d = nc.dram_tensor("ident", [128, 128], f32, kind="ExternalInput").ap()
    identb_d = nc.dram_tensor("identb", [128, 128], bf16, kind="ExternalInput").ap()
    wst_d = nc.dram_tensor("wst", [4, 128, K], bf16, kind="ExternalInput").ap()
    wl_d = nc.dram_tensor("wl", [4, K, K], bf16, kind="ExternalInput").ap()
    bh_d = nc.dram_tensor("bh", [DK, 16], f32, kind="ExternalInput").ap()
    bo_d = nc.dram_tensor("bo_col", [K, 1], f32, kind="ExternalInput").ap()
    ones_row_d = nc.dram_tensor("ones_row", [1, V], bf16, kind="ExternalInput").ap()
    ones_col_d = nc.dram_tensor("ones_col", [128, V // 128], bf16, kind="ExternalInput").ap()
    out_d = nc.dram_tensor("out_t", [K, VQ], f32, kind="ExternalOutput").ap()

    NVT = V // 128  # 16 vertex tiles

    with tile.TileContext(nc) as tc:
        with (
            tc.tile_pool(name="const", bufs=1) as cpool,
        ):
            # ---- persistent constants ----
            ident = cpool.tile([128, 128], f32)
            nc.sync.dma_start(ident[:], ident_d[:])
            identb = cpool.tile([128, 128], bf16)
            nc.sync.dma_start(identb[:], identb_d[:])
            sdt = cpool.tile([128, NCH, 128], bf16)
            nc.sync.dma_start(sdt[:], sdt_d.rearrange("c p m -> p c m"))
            wst = cpool.tile([128, 4, K], bf16)
            nc.sync.dma_start(wst[:], wst_d.rearrange("w a b -> a w b"))
            wl = cpool.tile([K, 4, K], bf16)
            nc.sync.dma_start(wl[:], wl_d.rearrange("w a b -> a w b"))
            bh = cpool.tile([DK, 16], f32)
            nc.sync.dma_start(bh[:], bh_d[:])
            bo = cpool.tile([K, 1], f32)
            nc.sync.dma_start(bo[:], bo_d[:])
            # persistent theta^T splits [h-rows 0:64 | l-rows 64:128]
            th_q = cpool.tile([128, VQ], bf16)
            th_k = cpool.tile([128, V], bf16)
            th_v = cpool.tile([128, V], bf16)
            # score operand tiles (x3 block layout), zeroed once; double-buffered
            # by head parity so head h+1's builds overlap head h's attention
            qa3_t, ka3_t, va_t = [], [], []
            for hb in range(2):
                qa3 = cpool.tile([96, VQ], bf16, name=f"qa3_{hb}")
                nc.vector.memset(qa3[:], 0.0)
                qa3_t.append(qa3)
                ka3 = cpool.tile([96, V], bf16, name=f"ka3_{hb}")
                nc.vector.memset(ka3[:], 0.0)
                nc.sync.dma_start(ka3[DK : DK + 1, :], ones_row_d[:])
                nc.sync.dma_start(ka3[64 + DK : 64 + DK + 1, :], ones_row_d[:])
                ka3_t.append(ka3)
                va = cpool.tile([128, V // 128, DK + 1], bf16, name=f"va_{hb}")
                nc.sync.dma_start(
                    va[:, :, DK : DK + 1].rearrange("p a b -> p (a b)"), ones_col_d[:]
                )
                va_t.append(va)
            O = cpool.tile([128, 8, K], f32)       # [128q, qt, 64]
            OT2 = cpool.tile([128, VQ], bf16)      # [OTh | OTl]
            outsb = cpool.tile([K, VQ], f32)
            # theta accumulation staging (reused per pr)
            xf = cpool.tile([K, V], f32)
            xcA = cpool.tile([K, V], f32)
            xcB = cpool.tile([K, V], f32)

            theta_stack = ExitStack()
            vtpool = theta_stack.enter_context(tc.tile_pool(name="vt", bufs=3))
            dxpool = theta_stack.enter_context(tc.tile_pool(name="dx", bufs=2))
            t4pool = theta_stack.enter_context(tc.tile_pool(name="t4p", bufs=1))
            pspool = theta_stack.enter_context(
                tc.tile_pool(name="ps", bufs=2, space="PSUM")
            )
            stpool = theta_stack.enter_context(tc.tile_pool(name="st", bufs=2))
            accpool = theta_stack.enter_context(tc.tile_pool(name="acc", bufs=2))

            t4s = []
            for g in range(4):
                t4_t = t4pool.tile([128, 8, 512], bf16, tag=f"t4_{g}", name=f"t4_{g}")
                t4s.append(t4_t)

            # ---- phase 1: per-vtile edge math -> dx staging -> DMA transposes ----
            for vt in range(NVT):
                g, vt4 = vt // 4, vt % 4
                vsl = slice(vt * 128, vt * 128 + 128)
                gath = vtpool.tile([128, N, 3], f32, tag="gath")
                nc.sync.dma_start(gath[:], gath_d[vsl, :, :])
                cent = vtpool.tile([128, 3], f32, tag="cent")
                nc.sync.dma_start(cent[:], verts_d[vsl, :])
                diff = vtpool.tile([128, N, 3], f32, tag="diff")
                for c in range(3):
                    nc.vector.tensor_tensor(
                        out=diff[:, :, c],
                        in0=gath[:, :, c],
                        in1=cent[:, c : c + 1].to_broadcast([128, N]),
                        op=Alu.subtract,
                    )
                dsq = vtpool.tile([128, N, 3], f32, tag="dsq")
                nc.scalar.square(dsq[:], diff[:])
                nsq = vtpool.tile([128, N], f32, tag="nsq")
                nc.vector.reduce_sum(nsq[:], dsq[:], axis=mybir.AxisListType.X)
                nrm = vtpool.tile([128, N], f32, tag="nrm")
                nc.scalar.sqrt(nrm[:], nsq[:])
                nc.vector.tensor_scalar_max(nrm[:], nrm[:], EPS)
                invn = vtpool.tile([128, N], f32, tag="invn")
                nc.vector.reciprocal(invn[:], nrm[:])
                tdn = vtpool.tile([128, N, 3], f32, tag="tdn")
                nc.vector.tensor_tensor(
                    out=tdn[:],
                    in0=diff[:],
                    in1=invn[:].to_broadcast([128, N, 3]),
                    op=Alu.mult,
                )
                # dx staging: [128, grp(8), slot(4), 32rows]; rows 0:3 = dirh,
                # rows 3:6 = dirl (neighbor n = 4*grp + slot)
                dx = dxpool.tile([128, 8, 4, 32], bf16, tag="dx")
                if vt < 2:
                    # zero the two rotating staging buffers once: garbage rows
                    # hit zero lhsT rows, and 0*NaN would poison the PSUM
                    nc.vector.memset(dx[:], 0.0)
                tdn_r = tdn[:].rearrange("p (g j) c -> p g j c", g=8)
                nc.vector.tensor_copy(dx[:, :, :, 0:3], tdn_r)
                nc.vector.tensor_tensor(
                    out=dx[:, :, :, 3:6], in0=tdn_r, in1=dx[:, :, :, 0:3],
                    op=Alu.subtract,
                )
                # batched xbar transpose: out[r, g8, v] = dx[v, 128*g8 + r]
                nc.sync.dma_start_transpose(
                    t4s[g][:, :, vt4 * 128 : vt4 * 128 + 128],
                    dx[:].rearrange("p a b c -> p (a b c)"),
                )

            # ---- phase 2: theta matmuls (4x row-tiled); relu+max; s-sum ----
            for pr in (1, 2, 0):  # k, v, q  (q last; th_q only needs own half)
                ngr = 2 if pr == 0 else 4
                for ch in range(2):
                    lhs = sdt[:, 2 * pr + ch, :]
                    for g in range(ngr):
                        acc = accpool.tile([128, 512], bf16, tag="acc")
                        for grp in range(8):
                            ps = pspool.tile([128, 2048], f32, tag="big")
                            for j in range(4):
                                nc.tensor.matmul(
                                    out=ps[:, 512 * j : 512 * j + 512],
                                    lhsT=lhs[32 * j : 32 * j + 32, :],
                                    rhs=t4s[g][32 * j : 32 * j + 32, grp, :],
                                    start=True,
                                    stop=True,
                                    tile_position=(32 * j, 0),
                                )
                            if grp < ACT_GRPS:
                                st = stpool.tile([128, 2048], bf16, tag="st")
                                nc.scalar.activation(st[:], ps[:], Act.Relu)
                                tmp = stpool.tile([128, 1024], bf16, tag="tmp")
                                nc.vector.tensor_tensor(
                                    out=tmp[:], in0=st[:, 0:1024],
                                    in1=st[:, 1024:2048], op=Alu.max,
                                )
                                if grp == 0:
                                    nc.vector.tensor_tensor(
                                        out=acc[:], in0=tmp[:, 0:512],
                                        in1=tmp[:, 512:1024], op=Alu.max,
                                    )
                                else:
                                    t2 = stpool.tile([128, 512], bf16, tag="t2")
                                    nc.vector.tensor_tensor(
                                        out=t2[:], in0=tmp[:, 0:512],
                                        in1=tmp[:, 512:1024], op=Alu.max,
                                    )
                                    nc.vector.tensor_tensor(
                                        out=acc[:], in0=t2[:], in1=acc[:],
                                        op=Alu.max,
                                    )
                            else:
                                for j in range(4):
                                    nc.vector.tensor_tensor(
                                        out=acc[:],
                                        in0=ps[:, 512 * j : 512 * j + 512],
                                        in1=acc[:],
                                        op=Alu.max,
                                    )
                        if ACT_GRPS < 8:
                            nc.vector.tensor_scalar_max(acc[:], acc[:], 0.0)
                        gsl = slice(g * 512, g * 512 + 512)
                        # DVE TT needs both SBUF inputs at the same base
                        # partition; shift the upper s-half down via DMA.
                        shp = accpool.tile([K, 512], bf16, tag="shp")
                        nc.sync.dma_start(shp[:], acc[K:128, :])
                        xc = xcA if ch == 0 else xcB
                        nc.vector.tensor_tensor(
                            out=xc[:, gsl], in0=acc[0:K, :], in1=shp[:],
                            op=Alu.add,
                        )
                        if ch == 1:
                            nc.vector.tensor_tensor(
                                out=xf[:, gsl], in0=xcA[:, gsl], in1=xcB[:, gsl],
                                op=Alu.add,
                            )
                # theta hi/lo split [128, VV]: rows 0:64 hi, 64:128 lo
                th = {0: th_q, 1: th_k, 2: th_v}[pr]
                vv = VQ if pr == 0 else V
                nc.vector.tensor_copy(th[0:K, :], xf[:, 0:vv])
                nc.vector.tensor_tensor(
                    out=th[K:128, :], in0=xf[:, 0:vv], in1=th[0:K, :],
                    op=Alu.subtract,
                )
            theta_stack.close()

            # ---- phase 3+4: per-head projection + attention ----
            attn_stack = ExitStack()
            atpool = attn_stack.enter_context(tc.tile_pool(name="attn", bufs=2))
            epool = attn_stack.enter_context(tc.tile_pool(name="epool", bufs=3))
            psxpool = attn_stack.enter_context(
                tc.tile_pool(name="psx", bufs=2, space="PSUM")
            )
            pstpool = attn_stack.enter_context(
                tc.tile_pool(name="pst", bufs=2, space="PSUM")
            )
            stppool = attn_stack.enter_context(
                tc.tile_pool(name="stp", bufs=2, space="PSUM")
            )

            # pre-attention: all heads' q/k projections and norm-bound rows,
            # so head boundaries don't serialize on them
            qfs, kfs, q4s, mrows = [], [], [], []
            for h in range(H):
                hsl = slice(DK * h, DK * h + DK)
                heads = {}
                for wi, (th, vv, nm) in enumerate(
                    ((th_q, VQ, "qf"), (th_k, V, "kf"))
                ):
                    hf = cpool.tile([DK, vv], f32, name=f"{nm}{h}")
                    heads[nm] = hf
                    for tt in range(vv // 512):
                        sl = slice(tt * 512, tt * 512 + 512)
                        pp = psxpool.tile([DK, 512], f32, tag="xps")
                        nc.tensor.matmul(
                            out=pp[:], lhsT=wst[:, wi, hsl], rhs=th[:, sl],
                            start=True, stop=False,
                        )
                        nc.tensor.matmul(
                            out=pp[:], lhsT=wl[:, wi, hsl], rhs=th[0:K, sl],
                            start=False, stop=True,
                        )
                        nc.scalar.activation(
                            hf[:, sl], pp[:], Act.Identity,
                            bias=bh[:, wi * 4 + h : wi * 4 + h + 1],
                        )
                qf, kf = heads["qf"], heads["kf"]
                qfs.append(qf)
                kfs.append(kf)
                q4 = cpool.tile([DK, VQ], f32, name=f"q4_{h}")
                nc.scalar.mul(q4[:], qf[:], 0.25)
                q4s.append(q4)
                # norm-bound shift: m = 4 * max|qf| * max|kf| (>= max score);
                # softmax is shift-invariant, exp(s-m) stays in [e^-2m, 1].
                qm = atpool.tile([DK, 1], f32, tag="qm")
                nc.vector.reduce_max(
                    qm[:], qf[:], axis=mybir.AxisListType.X,
                    apply_absolute_value=True,
                )
                km = atpool.tile([DK, 1], f32, tag="km")
                nc.vector.reduce_max(
                    km[:], kf[:], axis=mybir.AxisListType.X,
                    apply_absolute_value=True,
                )
                qmr = atpool.tile([1, DK], f32, tag="qmr")
                nc.sync.dma_start(qmr[:], qm[:].rearrange("p a -> a p"))
                kmr = atpool.tile([1, DK], f32, tag="kmr")
                nc.sync.dma_start(kmr[:], km[:].rearrange("p a -> a p"))
                qs1 = atpool.tile([1, 1], f32, tag="qs1")
                nc.vector.reduce_max(qs1[:], qmr[:], axis=mybir.AxisListType.X)
                ks1 = atpool.tile([1, 1], f32, tag="ks1")
                nc.vector.reduce_max(ks1[:], kmr[:], axis=mybir.AxisListType.X)
                ms = atpool.tile([1, 1], f32, tag="ms")
                nc.vector.tensor_tensor(
                    out=ms[:], in0=qs1[:], in1=ks1[:], op=Alu.mult
                )
                nc.vector.tensor_scalar_mul(ms[:], ms[:], -4.0)
                mrow = cpool.tile([1, VQ], bf16, name=f"mrow{h}")
                nc.vector.tensor_copy(mrow[:], ms[:].to_broadcast([1, VQ]))
                mrows.append(mrow)

            for h in range(H):
                hsl = slice(DK * h, DK * h + DK)
                qa3, ka3, va = qa3_t[h % 2], ka3_t[h % 2], va_t[h % 2]
                qf, kf, q4 = qfs[h], kfs[h], q4s[h]

                # v head, directly transposed: va[v, dk] accumulating hi/lo
                for hv in range(2):
                    psv = pstpool.tile([128, 128], f32, tag="small")
                    for c8 in range(8):
                        c = hv * 8 + c8
                        csl = slice(c * 128, c * 128 + 128)
                        osl = slice(16 * c8, 16 * c8 + 16)
                        nc.tensor.matmul(
                            out=psv[:, osl], lhsT=th_v[:, csl],
                            rhs=wst[:, 2, hsl],
                            start=(c8 == 0), stop=False,
                        )
                        nc.tensor.matmul(
                            out=psv[:, osl], lhsT=th_v[0:K, csl],
                            rhs=wl[:, 2, hsl],
                            start=False, stop=(c8 == 7),
                        )
                    nc.vector.tensor_copy(
                        va[:, hv * 8 : hv * 8 + 8, 0:DK],
                        psv[:].rearrange("p (a b) -> p a b", a=8),
                    )

                # ka3 blocks: [0:16]=kah, [32:48]=kal, [64:80]=kah
                nc.vector.tensor_copy(ka3[0:DK, :], kf[:])
                nc.vector.tensor_tensor(
                    out=ka3[32 : 32 + DK, :], in0=kf[:], in1=ka3[0:DK, :],
                    op=Alu.subtract,
                )
                nc.scalar.copy(ka3[64 : 64 + DK, :], ka3[0:DK, :])
                # qa3 blocks: [0:16]=qah, [32:48]=qah, [64:80]=qal (q/4)
                nc.vector.tensor_copy(qa3[0:DK, :], q4[:])
                nc.scalar.copy(qa3[32 : 32 + DK, :], qa3[0:DK, :])
                nc.vector.tensor_tensor(
                    out=qa3[64 : 64 + DK, :], in0=q4[:], in1=qa3[0:DK, :],
                    op=Alu.subtract,
                )
                nc.sync.dma_start(qa3[DK : DK + 1, :], mrows[h][:])

                # ST' + exp + PV, software-pipelined: each ST' pair is issued
                # one step ahead of the matching PV pair so the in-order PE
                # queue never stalls behind a PV that waits on its exp.
                NK2 = V // 256
                for qs in range(VQ // 512):
                    pv = psxpool.tile([DK + 1, 512], f32, tag="xps")

                    def emit_st(k2):
                        stp = stppool.tile([128, 1024], f32, tag="stp", name="stp")
                        for kk in range(2):
                            kt = k2 * 2 + kk
                            nc.tensor.matmul(
                                out=stp[:, 512 * kk : 512 * kk + 512],
                                lhsT=ka3[0:AUG, kt * 128 : kt * 128 + 128],
                                rhs=qa3[0:AUG, qs * 512 : qs * 512 + 512],
                                start=True,
                                stop=True,
                            )
                        return stp

                    stp_cur = emit_st(0)
                    for k2 in range(NK2):
                        e = epool.tile([128, 1024], bf16, tag="e")
                        nc.scalar.activation(e[:], stp_cur[:], Act.Exp)
                        if k2 + 1 < NK2:
                            stp_cur = emit_st(k2 + 1)
                        for kk in range(2):
                            kt = k2 * 2 + kk
                            nc.tensor.matmul(
                                out=pv[:],
                                lhsT=va[:, kt, :],
                                rhs=e[:, 512 * kk : 512 * kk + 512],
                                start=(kt == 0),
                                stop=(kt == V // 128 - 1),
                            )
                    pvs = atpool.tile([DK + 1, 512], f32, tag="pvs")
                    nc.vector.tensor_copy(pvs[:], pv[:])
                    for q4i in range(4):
                        qt = qs * 4 + q4i
                        pq = pstpool.tile([128, DK + 1], f32, tag="small")
                        nc.tensor.transpose(
                            pq[:], pvs[:, q4i * 128 : q4i * 128 + 128],
                            ident[0 : DK + 1, 0 : DK + 1],
                        )
                        rz = atpool.tile([128, 1], f32, tag="rz")
                        nc.vector.reciprocal(rz[:], pq[:, DK : DK + 1])
                        nc.vector.tensor_scalar_mul(O[:, qt, hsl], pq[:, 0:DK], rz[:])

            # ---- phase 5: O hi/lo transpose + final projection ----
            for qt in range(8):
                qsl = slice(qt * 128, qt * 128 + 128)
                oh = atpool.tile([128, K], bf16, tag="oh")
                nc.vector.tensor_copy(oh[:], O[:, qt, :])
                ol = atpool.tile([128, K], bf16, tag="ol")
                nc.vector.tensor_tensor(
                    out=ol[:], in0=O[:, qt, :], in1=oh[:], op=Alu.subtract
                )
                oph = pstpool.tile([K, 128], bf16, tag="small")
                nc.tensor.transpose(oph[:], oh[:], identb[:])
                nc.scalar.copy(OT2[0:K, qsl], oph[:])
                opl = pstpool.tile([K, 128], bf16, tag="small")
                nc.tensor.transpose(opl[:], ol[:], identb[:])
                nc.scalar.copy(OT2[K:128, qsl], opl[:])
            for qs in range(VQ // 512):
                sl = slice(qs * 512, qs * 512 + 512)
                fp = psxpool.tile([K, 512], f32, tag="xps")
                nc.tensor.matmul(
                    out=fp[:], lhsT=wst[:, 3, :], rhs=OT2[:, sl],
                    start=True, stop=False,
                )
                nc.tensor.matmul(
                    out=fp[:], lhsT=wl[:, 3, :], rhs=OT2[0:K, sl],
                    start=False, stop=True,
                )
                nc.scalar.activation(outsb[:, sl], fp[:], Act.Identity, bias=bo[:])
            nc.sync.dma_start(out_d[:], outsb[:])
            attn_stack.close()

    nc.compile()
    return nc


def _host_prep(inputs):
    """Build the 8 per-core input maps from full inputs."""
    import ml_dtypes

    bfd = ml_dtypes.bfloat16
    verts = np.ascontiguousarray(np.asarray(inputs["vertices"], dtype=np.float32))
    idx = np.ascontiguousarray(np.asarray(inputs["neighbor_index"]).astype(np.int32))

    sd = np.concatenate(
        [np.asarray(inputs["q_dirs"]), np.asarray(inputs["k_dirs"]),
         np.asarray(inputs["v_dirs"])], axis=1
    ).astype(np.float32)  # [3, 768]
    nrm = np.sqrt((sd * sd).sum(0, dtype=np.float32), dtype=np.float32)
    sdn = (sd / np.maximum(nrm, np.float32(EPS))).astype(np.float32)
    sdh = sdn.astype(bfd)

    # dense lhsT bank: [ch, 128, 128]; per 32-row slot j: rows 0:3 and 3:6
    # both carry sdh for the chunk's 128 sk columns (pairs with dirh; dirl)
    sdt = np.zeros((NCH, 128, 128), bfd)
    for ch in range(NCH):
        blk = sdh[:, ch * 128 : ch * 128 + 128]
        for j in range(4):
            sdt[ch, 32 * j + 0 : 32 * j + 3, :] = blk
            sdt[ch, 32 * j + 3 : 32 * j + 6, :] = blk

    # weights: wst [4, 128, 64] = [Wh.T ; Wh.T], wl [4, 64, 64] = Wl.T
    wst = np.zeros((4, 128, K), bfd)
    wlo = np.zeros((4, K, K), bfd)
    for wi, kk in enumerate(("Wq", "Wk", "Wv", "Wo")):
        wt_ = np.asarray(inputs[kk], dtype=np.float32).T
        wh_ = wt_.astype(bfd)
        wst[wi, 0:K, :] = wh_
        wst[wi, K:128, :] = wh_
        wlo[wi] = (wt_ - wh_.astype(np.float32)).astype(bfd)

    bh = np.zeros((DK, 16), np.float32)
    for wi, kk in enumerate(("bq", "bk", "bv", "bo")):
        bb_ = np.asarray(inputs[kk], dtype=np.float32)
        for h in range(H):
            bh[:, wi * 4 + h] = bb_[DK * h : DK * h + DK]
    # bv folded into output bias: attention rows sum to 1, so the +bv inside
    # vh passes through to x additively; x@Wo.T + bo == pv@Wo.T + (Wo@bv + bo)
    bo2 = (
        np.asarray(inputs["bo"], dtype=np.float32)
        + np.asarray(inputs["Wo"], dtype=np.float32)
        @ np.asarray(inputs["bv"], dtype=np.float32)
    ).reshape(K, 1)

    common = {
        "sdt": sdt,
        "ident": np.eye(128, dtype=np.float32),
        "identb": np.eye(128, dtype=np.float32).astype(bfd),
        "wst": wst,
        "wl": wlo,
        "bh": bh,
        "bo_col": bo2,
        "ones_row": np.ones((1, V), bfd),
        "ones_col": np.ones((128, V // 128), bfd),
    }

    in_maps = []
    for core in range(8):
        bb, half = core // 2, core % 2
        if half == 0:
            vb, ib = verts[bb], idx[bb]
        else:
            perm = np.concatenate([np.arange(VQ, V), np.arange(0, VQ)])
            vb = verts[bb][perm]
            ib = np.where(idx[bb][perm] >= VQ, idx[bb][perm] - VQ, idx[bb][perm] + VQ)
        in_maps.append({
            "verts": np.ascontiguousarray(vb),
            "gath": np.ascontiguousarray(vb[ib]),
            **common,
        })
    return in_maps


def run(inputs, trace=False, trace_kwargs=None):
    from concourse.bass_utils import run_bass_kernel_spmd

    if "nc" not in _CACHE:
        _CACHE["nc"] = _build_program()
    nc = _CACHE["nc"]
    in_maps = _host_prep(inputs)
    res = run_bass_kernel_spmd(
        nc, in_maps, core_ids=list(range(8)), trace=trace,
        **(trace_kwargs or {}),
    )
    out = np.zeros((BS, V, K), np.float32)
    for core in range(8):
        bb, half = core // 2, core % 2
        ot = res.results[core]["out_t"]  # [64, 1024]
        out[bb, half * VQ : half * VQ + VQ, :] = ot.T
    return out, res


def kernel(**inputs) -> np.ndarray:
    out, _ = run(inputs, trace=False)
    return out


# revision 20
# speedup vs baseline: 1.4630x; 1.3022x over previous
"""Trainium2 Bass kernel for nn_Attention_Conv_surface (gnn_message_passing).

Math (per batch b):
  neighbors = vertices[idx]                          # (V, N, 3)
  dirn = normalize(neighbors - vertices[:, None])    # (V, N, 3)
  theta_d = sum_s max_n relu(dirn @ sdn_d)           # (V, K) for d in {q,k,v}
  qkv = theta @ W.T + b ; MHA over full VxV ; out = attn_out @ Wo.T + bo

Device strategy (v2):
  * theta matmuls use 4x row-tiling (tile_position): 4 neighbors run
    concurrently on 32-row PE tiles.  Per neighbor the contraction is 6 rows:
    (dirh; dirl) against (sdh; sdh), i.e. sdh * dirn_fp32 exactly; only the
    support-direction rounding (~2^-9, fixed per sk column) remains.
  * t4 operand layout [128 rows, grp, 512v] is built with DMA-xbar transposes
    (no PE/ACT involvement); rows 6..31 of each 32-row slot are garbage and
    are killed by zero rows in the dense per-chunk lhsT.
  * max over n: 4-neighbor PSUM supertiles [128, 2048]; most groups take the
    ACT route (fused relu + bf16 cast to SBUF, then bf16 2x tensor_tensor max
    on DVE), the rest are maxed straight out of PSUM on DVE.  The ACT/DVE
    ratio (ACT_GRPS) balances the two engines.
  * attention: scores transposed with augmented operands qa=[qh/4;-m], ka=[kh;1]
    in an x3 block layout; m is a per-head norm bound (4*max|qf|*max|kf|),
    valid since softmax is shift-invariant and exp(s-m) stays in [e^-2m, 1].
  * v head is produced directly transposed: va[v,dk] via matmuls with
    lhsT=theta_v columns, accumulating the Wv hi/lo product; bv is folded into
    the output bias on the host (exact, since attention rows sum to 1).
  * exp on ACT in [128,1024] two-bank batches; PV augments v with a ones row
    so the softmax denominator falls out of the same matmul.

Sharding: 8 cores = (batch 0..3) x (query half 0..1).  Each core computes
k/v thetas for the full batch (duplicated within the pair) and q theta +
attention for its own 1024 queries.  Identical SPMD program; the query half is
selected by feeding each core a half-rolled permutation of its batch's data.
"""

import numpy as np

BS, V, N, S, K, H = 4, 2048, 32, 4, 64, 4
DK = K // H
VQ = V // 2          # queries per core
NCH = 6              # sk chunks of 128 (768 total = 3 dirs * 256)
EPS = 1e-12
AUG = 81             # rows used of the x3-block score operands
ACT_GRPS = 7         # of 8 neighbor-groups per (ch,g): routed via ACT relu-copy

_CACHE = {}


def _build_program():
    import concourse.bass as bass
    import concourse.mybir as mybir
    import concourse.tile as tile
    from concourse import bacc
    from contextlib import ExitStack

    f32 = mybir.dt.float32
    bf16 = mybir.dt.bfloat16
    Alu = mybir.AluOpType
    Act = mybir.ActivationFunctionType

    nc = bacc.Bacc("TRN2", target_bir_lowering=False, debug=False, num_devices=8)

    # ---- DRAM I/O ----
    verts_d = nc.dram_tensor("verts", [V, 3], f32, kind="ExternalInput").ap()
    gath_d = nc.dram_tensor("gath", [V, N, 3], f32, kind="ExternalInput").ap()
    sdt_d = nc.dram_tensor("sdt", [NCH, 128, 128], bf16, kind="ExternalInput").ap()
    ident_d = nc.dram_tensor("ident", [128, 128], f32, kind="ExternalInput").ap()
    identb_d = nc.dram_tensor("identb", [128, 128], bf16, kind="ExternalInput").ap()
    wst_d = nc.dram_tensor("wst", [4, 128, K], bf16, kind="ExternalInput").ap()
    wl_d = nc.dram_tensor("wl", [4, K, K], bf16, kind="ExternalInput").ap()
    bh_d = nc.dram_tensor("bh", [DK, 16], f32, kind="ExternalInput").ap()
    bo_d = nc.dram_tensor("bo_col", [K, 1], f32, kind="ExternalInput").ap()
    ones_row_d = nc.dram_tensor("ones_row", [1, V], bf16, kind="ExternalInput").ap()
    ones_col_d = nc.dram_tensor("ones_col", [128, V // 128], bf16, kind="ExternalInput").ap()
    out_d = nc.dram_tensor("out_t", [K, VQ], f32, kind="ExternalOutput").ap()
    # k/v theta halves are exchanged across the core pair via AllGather
    cc_in = {
        pr: nc.dram_tensor(f"cc_in_{pr}", [128, VQ], bf16, kind="Internal").ap()
        for pr in (1, 2)
    }
    cc_out = {
        pr: nc.dram_tensor(f"cc_out_{pr}", [2, 128, VQ], bf16, kind="Internal").ap()
        for pr in (1, 2)
    }

    NVT = V // 128  # 16 vertex tiles

    with tile.TileContext(nc) as tc:
        with (
            tc.tile_pool(name="const", bufs=1) as cpool,
        ):
            # ---- persistent constants ----
            ident = cpool.tile([128, 128], f32)
            nc.sync.dma_start(ident[:], ident_d[:])
            identb = cpool.tile([128, 128], bf16)
            nc.sync.dma_start(identb[:], identb_d[:])
            sdt = cpool.tile([128, NCH, 128], bf16)
            nc.sync.dma_start(sdt[:], sdt_d.rearrange("c p m -> p c m"))
            wst = cpool.tile([128, 4, K], bf16)
            nc.sync.dma_start(wst[:], wst_d.rearrange("w a b -> a w b"))
            wl = cpool.tile([K, 4, K], bf16)
            nc.sync.dma_start(wl[:], wl_d.rearrange("w a b -> a w b"))
            bh = cpool.tile([DK, 16], f32)
            nc.sync.dma_start(bh[:], bh_d[:])
            bo = cpool.tile([K, 1], f32)
            nc.sync.dma_start(bo[:], bo_d[:])
            # persistent theta^T splits [h-rows 0:64 | l-rows 64:128]
            th_q = cpool.tile([128, VQ], bf16)
            th_k = cpool.tile([128, V], bf16)
            th_v = cpool.tile([128, V], bf16)
            # score operand tiles (x3 block layout), zeroed once; double-buffered
            # by head parity so head h+1's builds overlap head h's attention
            qa3_t, ka3_t, va_t = [], [], []
            for hb in range(2):
                qa3 = cpool.tile([96, VQ], bf16, name=f"qa3_{hb}")
                nc.vector.memset(qa3[:], 0.0)
                qa3_t.append(qa3)
                ka3 = cpool.tile([96, V], bf16, name=f"ka3_{hb}")
                nc.vector.memset(ka3[:], 0.0)
                nc.sync.dma_start(ka3[DK : DK + 1, :], ones_row_d[:])
                nc.sync.dma_start(ka3[64 + DK : 64 + DK + 1, :], ones_row_d[:])
                ka3_t.append(ka3)
                va = cpool.tile([128, V // 128, DK + 1], bf16, name=f"va_{hb}")
                nc.sync.dma_start(
                    va[:, :, DK : DK + 1].rearrange("p a b -> p (a b)"), ones_col_d[:]
                )
                va_t.append(va)
            O = cpool.tile([128, 8, K], f32)       # [128q, qt, 64]
            OT2 = cpool.tile([128, VQ], bf16)      # [OTh | OTl]
            outsb = cpool.tile([K, VQ], f32)
            # theta accumulation staging (reused per pr)
            xf = cpool.tile([K, V], f32)
            xcA = cpool.tile([K, V], f32)
            xcB = cpool.tile([K, V], f32)

            theta_stack = ExitStack()
            vtpool = theta_stack.enter_context(tc.tile_pool(name="vt", bufs=3))
            dxpool = theta_stack.enter_context(tc.tile_pool(name="dx", bufs=2))
            t4pool = theta_stack.enter_context(tc.tile_pool(name="t4p", bufs=1))
            pspool = theta_stack.enter_context(
                tc.tile_pool(name="ps", bufs=2, space="PSUM")
            )
            stpool = theta_stack.enter_context(tc.tile_pool(name="st", bufs=2))
            accpool = theta_stack.enter_context(tc.tile_pool(name="acc", bufs=2))

            t4s = []
            for g in range(4):
                t4_t = t4pool.tile([128, 8, 512], bf16, tag=f"t4_{g}", name=f"t4_{g}")
                t4s.append(t4_t)

            # ---- phase 1: per-vtile edge math -> dx staging -> DMA transposes ----
            for vt in range(NVT):
                g, vt4 = vt // 4, vt % 4
                vsl = slice(vt * 128, vt * 128 + 128)
                gath = vtpool.tile([128, N, 3], f32, tag="gath")
                nc.sync.dma_start(gath[:], gath_d[vsl, :, :])
                cent = vtpool.tile([128, 3], f32, tag="cent")
                nc.sync.dma_start(cent[:], verts_d[vsl, :])
                diff = vtpool.tile([128, N, 3], f32, tag="diff")
                for c in range(3):
                    nc.vector.tensor_tensor(
                        out=diff[:, :, c],
                        in0=gath[:, :, c],
                        in1=cent[:, c : c + 1].to_broadcast([128, N]),
                        op=Alu.subtract,
                    )
                dsq = vtpool.tile([128, N, 3], f32, tag="dsq")
                nc.scalar.square(dsq[:], diff[:])
                nsq = vtpool.tile([128, N], f32, tag="nsq")
                nc.vector.reduce_sum(nsq[:], dsq[:], axis=mybir.AxisListType.X)
                nrm = vtpool.tile([128, N], f32, tag="nrm")
                nc.scalar.sqrt(nrm[:], nsq[:])
                nc.vector.tensor_scalar_max(nrm[:], nrm[:], EPS)
                invn = vtpool.tile([128, N], f32, tag="invn")
                nc.vector.reciprocal(invn[:], nrm[:])
                tdn = vtpool.tile([128, N, 3], f32, tag="tdn")
                nc.vector.tensor_tensor(
                    out=tdn[:],
                    in0=diff[:],
                    in1=invn[:].to_broadcast([128, N, 3]),
                    op=Alu.mult,
                )
                # dx staging: [128, grp(8), slot(4), 32rows]; rows 0:3 = dirh,
                # rows 3:6 = dirl (neighbor n = 4*grp + slot)
                dx = dxpool.tile([128, 8, 4, 32], bf16, tag="dx")
                if vt < 2:
                    # zero the two rotating staging buffers once: garbage rows
                    # hit zero lhsT rows, and 0*NaN would poison the PSUM
                    nc.vector.memset(dx[:], 0.0)
                tdn_r = tdn[:].rearrange("p (g j) c -> p g j c", g=8)
                nc.vector.tensor_copy(dx[:, :, :, 0:3], tdn_r)
                nc.vector.tensor_tensor(
                    out=dx[:, :, :, 3:6], in0=tdn_r, in1=dx[:, :, :, 0:3],
                    op=Alu.subtract,
                )
                # batched xbar transpose: out[r, g8, v] = dx[v, 128*g8 + r]
                nc.sync.dma_start_transpose(
                    t4s[g][:, :, vt4 * 128 : vt4 * 128 + 128],
                    dx[:].rearrange("p a b c -> p (a b c)"),
                )

            # ---- phase 2: theta matmuls (4x row-tiled); relu+max; s-sum ----
            # every direction computes its OWN half only (g in {0,1}); k/v
            # full-V thetas are assembled by a pair AllGather afterwards
            for pr in (1, 2, 0):  # k, v, q
                ngr = 2
                for ch in range(2):
                    lhs = sdt[:, 2 * pr + ch, :]
                    for g in range(ngr):
                        acc = accpool.tile([128, 512], bf16, tag="acc")
                        for grp in range(8):
                            ps = pspool.tile([128, 2048], f32, tag="big")
                            for j in range(4):
                                nc.tensor.matmul(
                                    out=ps[:, 512 * j : 512 * j + 512],
                                    lhsT=lhs[32 * j : 32 * j + 32, :],
                                    rhs=t4s[g][32 * j : 32 * j + 32, grp, :],
                                    start=True,
                                    stop=True,
                                    tile_position=(32 * j, 0),
                                )
                            if grp < ACT_GRPS:
                                st = stpool.tile([128, 2048], bf16, tag="st")
                                nc.scalar.activation(st[:], ps[:], Act.Relu)
                                tmp = stpool.tile([128, 1024], bf16, tag="tmp")
                                nc.vector.tensor_tensor(
                                    out=tmp[:], in0=st[:, 0:1024],
                                    in1=st[:, 1024:2048], op=Alu.max,
                                )
                                if grp == 0:
                                    nc.vector.tensor_tensor(
                                        out=acc[:], in0=tmp[:, 0:512],
                                        in1=tmp[:, 512:1024], op=Alu.max,
                                    )
                                else:
                                    t2 = stpool.tile([128, 512], bf16, tag="t2")
                                    nc.vector.tensor_tensor(
                                        out=t2[:], in0=tmp[:, 0:512],
                                        in1=tmp[:, 512:1024], op=Alu.max,
                                    )
                                    nc.vector.tensor_tensor(
                                        out=acc[:], in0=t2[:], in1=acc[:],
                                        op=Alu.max,
                                    )
                            else:
                                for j in range(4):
                                    nc.vector.tensor_tensor(
                                        out=acc[:],
                                        in0=ps[:, 512 * j : 512 * j + 512],
                                        in1=acc[:],
                                        op=Alu.max,
                                    )
                        if ACT_GRPS < 8:
                            nc.vector.tensor_scalar_max(acc[:], acc[:], 0.0)
                        gsl = slice(g * 512, g * 512 + 512)
                        # DVE TT needs both SBUF inputs at the same base
                        # partition; shift the upper s-half down via DMA.
                        shp = accpool.tile([K, 512], bf16, tag="shp")
                        nc.sync.dma_start(shp[:], acc[K:128, :])
                        xc = xcA if ch == 0 else xcB
                        nc.vector.tensor_tensor(
                            out=xc[:, gsl], in0=acc[0:K, :], in1=shp[:],
                            op=Alu.add,
                        )
                        if ch == 1:
                            nc.vector.tensor_tensor(
                                out=xf[:, gsl], in0=xcA[:, gsl], in1=xcB[:, gsl],
                                op=Alu.add,
                            )
                # theta hi/lo split of the own half: rows 0:64 hi, 64:128 lo
                th = {0: th_q, 1: th_k, 2: th_v}[pr]
                nc.vector.tensor_copy(th[0:K, 0:VQ], xf[:, 0:VQ])
                nc.vector.tensor_tensor(
                    out=th[K:128, 0:VQ], in0=xf[:, 0:VQ], in1=th[0:K, 0:VQ],
                    op=Alu.subtract,
                )
                if pr != 0:
                    # exchange halves across the pair; ranks concat in global
                    # v-order, which is what attention k/v consumers expect
                    nc.sync.dma_start(cc_in[pr][:], th[:, 0:VQ])
                    nc.gpsimd.collective_compute(
                        "AllGather",
                        mybir.AluOpType.bypass,
                        replica_groups=[[0, 1], [2, 3], [4, 5], [6, 7]],
                        ins=[cc_in[pr][:]],
                        outs=[cc_out[pr][:]],
                    )
                    nc.sync.dma_start(
                        th[:].rearrange("p (r v) -> p r v", r=2),
                        cc_out[pr].rearrange("r p v -> p r v"),
                    )
            theta_stack.close()

            # ---- phase 3+4: per-head projection + attention ----
            attn_stack = ExitStack()
            atpool = attn_stack.enter_context(tc.tile_pool(name="attn", bufs=2))
            epool = attn_stack.enter_context(tc.tile_pool(name="epool", bufs=3))
            psxpool = attn_stack.enter_context(
                tc.tile_pool(name="psx", bufs=2, space="PSUM")
            )
            pstpool = attn_stack.enter_context(
                tc.tile_pool(name="pst", bufs=2, space="PSUM")
            )
            stppool = attn_stack.enter_context(
                tc.tile_pool(name="stp", bufs=2, space="PSUM")
            )

            # pre-attention: all heads' q/k projections and norm-bound rows,
            # so head boundaries don't serialize on them
            qfs, kfs, q4s, mrows = [], [], [], []
            for h in range(H):
                hsl = slice(DK * h, DK * h + DK)
                heads = {}
                for wi, (th, vv, nm) in enumerate(
                    ((th_q, VQ, "qf"), (th_k, V, "kf"))
                ):
                    hf = cpool.tile([DK, vv], f32, name=f"{nm}{h}")
                    heads[nm] = hf
                    for tt in range(vv // 512):
                        sl = slice(tt * 512, tt * 512 + 512)
                        pp = psxpool.tile([DK, 512], f32, tag="xps")
                        nc.tensor.matmul(
                            out=pp[:], lhsT=wst[:, wi, hsl], rhs=th[:, sl],
                            start=True, stop=False,
                        )
                        nc.tensor.matmul(
                            out=pp[:], lhsT=wl[:, wi, hsl], rhs=th[0:K, sl],
                            start=False, stop=True,
                        )
                        nc.scalar.activation(
                            hf[:, sl], pp[:], Act.Identity,
                            bias=bh[:, wi * 4 + h : wi * 4 + h + 1],
                        )
                qf, kf = heads["qf"], heads["kf"]
                qfs.append(qf)
                kfs.append(kf)
                q4 = cpool.tile([DK, VQ], f32, name=f"q4_{h}")
                nc.scalar.mul(q4[:], qf[:], 0.25)
                q4s.append(q4)
                # norm-bound shift: m = 4 * max|qf| * max|kf| (>= max score);
                # softmax is shift-invariant, exp(s-m) stays in [e^-2m, 1].
                qm = atpool.tile([DK, 1], f32, tag="qm")
                nc.vector.reduce_max(
                    qm[:], qf[:], axis=mybir.AxisListType.X,
                    apply_absolute_value=True,
                )
                km = atpool.tile([DK, 1], f32, tag="km")
                nc.vector.reduce_max(
                    km[:], kf[:], axis=mybir.AxisListType.X,
                    apply_absolute_value=True,
                )
                qmr = atpool.tile([1, DK], f32, tag="qmr")
                nc.sync.dma_start(qmr[:], qm[:].rearrange("p a -> a p"))
                kmr = atpool.tile([1, DK], f32, tag="kmr")
                nc.sync.dma_start(kmr[:], km[:].rearrange("p a -> a p"))
                qs1 = atpool.tile([1, 1], f32, tag="qs1")
                nc.vector.reduce_max(qs1[:], qmr[:], axis=mybir.AxisListType.X)
                ks1 = atpool.tile([1, 1], f32, tag="ks1")
                nc.vector.reduce_max(ks1[:], kmr[:], axis=mybir.AxisListType.X)
                ms = atpool.tile([1, 1], f32, tag="ms")
                nc.vector.tensor_tensor(
                    out=ms[:], in0=qs1[:], in1=ks1[:], op=Alu.mult
                )
                nc.vector.tensor_scalar_mul(ms[:], ms[:], -4.0)
                mrow = cpool.tile([1, VQ], bf16, name=f"mrow{h}")
                nc.vector.tensor_copy(mrow[:], ms[:].to_broadcast([1, VQ]))
                mrows.append(mrow)

            for h in range(H):
                hsl = slice(DK * h, DK * h + DK)
                qa3, ka3, va = qa3_t[h % 2], ka3_t[h % 2], va_t[h % 2]
                qf, kf, q4 = qfs[h], kfs[h], q4s[h]

                # v head, directly transposed: va[v, dk] accumulating hi/lo
                for hv in range(2):
                    psv = pstpool.tile([128, 128], f32, tag="small")
                    for c8 in range(8):
                        c = hv * 8 + c8
                        csl = slice(c * 128, c * 128 + 128)
                        osl = slice(16 * c8, 16 * c8 + 16)
                        nc.tensor.matmul(
                            out=psv[:, osl], lhsT=th_v[:, csl],
                            rhs=wst[:, 2, hsl],
                            start=(c8 == 0), stop=False,
                        )
                        nc.tensor.matmul(
                            out=psv[:, osl], lhsT=th_v[0:K, csl],
                            rhs=wl[:, 2, hsl],
                            start=False, stop=(c8 == 7),
                        )
                    nc.vector.tensor_copy(
                        va[:, hv * 8 : hv * 8 + 8, 0:DK],
                        psv[:].rearrange("p (a b) -> p a b", a=8),
                    )

                # ka3 blocks: [0:16]=kah, [32:48]=kal, [64:80]=kah
                nc.vector.tensor_copy(ka3[0:DK, :], kf[:])
                nc.vector.tensor_tensor(
                    out=ka3[32 : 32 + DK, :], in0=kf[:], in1=ka3[0:DK, :],
                    op=Alu.subtract,
                )
                nc.scalar.copy(ka3[64 : 64 + DK, :], ka3[0:DK, :])
                # qa3 blocks: [0:16]=qah, [32:48]=qah, [64:80]=qal (q/4)
                nc.vector.tensor_copy(qa3[0:DK, :], q4[:])
                nc.scalar.copy(qa3[32 : 32 + DK, :], qa3[0:DK, :])
                nc.vector.tensor_tensor(
                    out=qa3[64 : 64 + DK, :], in0=q4[:], in1=qa3[0:DK, :],
                    op=Alu.subtract,
                )
                nc.sync.dma_start(qa3[DK : DK + 1, :], mrows[h][:])

                # ST' + exp + PV, software-pipelined: each ST' pair is issued
                # one step ahead of the matching PV pair so the in-order PE
                # queue never stalls behind a PV that waits on its exp.
                NK2 = V // 256
                for qs in range(VQ // 512):
                    pv = psxpool.tile([DK + 1, 512], f32, tag="xps")

                    def emit_st(k2):
                        stp = stppool.tile([128, 1024], f32, tag="stp", name="stp")
                        for kk in range(2):
                            kt = k2 * 2 + kk
                            nc.tensor.matmul(
                                out=stp[:, 512 * kk : 512 * kk + 512],
                                lhsT=ka3[0:AUG, kt * 128 : kt * 128 + 128],
                                rhs=qa3[0:AUG, qs * 512 : qs * 512 + 512],
                                start=True,
                                stop=True,
                            )
                        return stp

                    stp_cur = emit_st(0)
                    for k2 in range(NK2):
                        e = epool.tile([128, 1024], bf16, tag="e")
                        nc.scalar.activation(e[:], stp_cur[:], Act.Exp)
                        if k2 + 1 < NK2:
                            stp_cur = emit_st(k2 + 1)
                        for kk in range(2):
                            kt = k2 * 2 + kk
                            nc.tensor.matmul(
                                out=pv[:],
                                lhsT=va[:, kt, :],
                                rhs=e[:, 512 * kk : 512 * kk + 512],
                                start=(kt == 0),
                                stop=(kt == V // 128 - 1),
                            )
                    pvs = atpool.tile([DK + 1, 512], f32, tag="pvs")
                    nc.vector.tensor_copy(pvs[:], pv[:])
                    for q4i in range(4):
                        qt = qs * 4 + q4i
                        pq = pstpool.tile([128, DK + 1], f32, tag="small")
                        nc.tensor.transpose(
                            pq[:], pvs[:, q4i * 128 : q4i * 128 + 128],
                            ident[0 : DK + 1, 0 : DK + 1],
                        )
                        rz = atpool.tile([128, 1], f32, tag="rz")
                        nc.vector.reciprocal(rz[:], pq[:, DK : DK + 1])
                        nc.vector.tensor_scalar_mul(O[:, qt, hsl], pq[:, 0:DK], rz[:])

            # ---- phase 5: O hi/lo transpose + final projection ----
            for qt in range(8):
                qsl = slice(qt * 128, qt * 128 + 128)
                oh = atpool.tile([128, K], bf16, tag="oh")
                nc.vector.tensor_copy(oh[:], O[:, qt, :])
                ol = atpool.tile([128, K], bf16, tag="ol")
                nc.vector.tensor_tensor(
                    out=ol[:], in0=O[:, qt, :], in1=oh[:], op=Alu.subtract
                )
                oph = pstpool.tile([K, 128], bf16, tag="small")
                nc.tensor.transpose(oph[:], oh[:], identb[:])
                nc.scalar.copy(OT2[0:K, qsl], oph[:])
                opl = pstpool.tile([K, 128], bf16, tag="small")
                nc.tensor.transpose(opl[:], ol[:], identb[:])
                nc.scalar.copy(OT2[K:128, qsl], opl[:])
            for qs in range(VQ // 512):
                sl = slice(qs * 512, qs * 512 + 512)
                fp = psxpool.tile([K, 512], f32, tag="xps")
                nc.tensor.matmul(
                    out=fp[:], lhsT=wst[:, 3, :], rhs=OT2[:, sl],
                    start=True, stop=False,
                )
                nc.tensor.matmul(
                    out=fp[:], lhsT=wl[:, 3, :], rhs=OT2[0:K, sl],
                    start=False, stop=True,
                )
                nc.scalar.activation(outsb[:, sl], fp[:], Act.Identity, bias=bo[:])
            nc.sync.dma_start(out_d[:], outsb[:])
            attn_stack.close()

    nc.compile()
    return nc


def _host_prep(inputs):
    """Build the 8 per-core input maps from full inputs."""
    import ml_dtypes

    bfd = ml_dtypes.bfloat16
    verts = np.ascontiguousarray(np.asarray(inputs["vertices"], dtype=np.float32))
    idx = np.ascontiguousarray(np.asarray(inputs["neighbor_index"]).astype(np.int32))

    sd = np.concatenate(
        [np.asarray(inputs["q_dirs"]), np.asarray(inputs["k_dirs"]),
         np.asarray(inputs["v_dirs"])], axis=1
    ).astype(np.float32)  # [3, 768]
    nrm = np.sqrt((sd * sd).sum(0, dtype=np.float32), dtype=np.float32)
    sdn = (sd / np.maximum(nrm, np.float32(EPS))).astype(np.float32)
    sdh = sdn.astype(bfd)

    # dense lhsT bank: [ch, 128, 128]; per 32-row slot j: rows 0:3 and 3:6
    # both carry sdh for the chunk's 128 sk columns (pairs with dirh; dirl)
    sdt = np.zeros((NCH, 128, 128), bfd)
    for ch in range(NCH):
        blk = sdh[:, ch * 128 : ch * 128 + 128]
        for j in range(4):
            sdt[ch, 32 * j + 0 : 32 * j + 3, :] = blk
            sdt[ch, 32 * j + 3 : 32 * j + 6, :] = blk

    # weights: wst [4, 128, 64] = [Wh.T ; Wh.T], wl [4, 64, 64] = Wl.T
    wst = np.zeros((4, 128, K), bfd)
    wlo = np.zeros((4, K, K), bfd)
    for wi, kk in enumerate(("Wq", "Wk", "Wv", "Wo")):
        wt_ = np.asarray(inputs[kk], dtype=np.float32).T
        wh_ = wt_.astype(bfd)
        wst[wi, 0:K, :] = wh_
        wst[wi, K:128, :] = wh_
        wlo[wi] = (wt_ - wh_.astype(np.float32)).astype(bfd)

    bh = np.zeros((DK, 16), np.float32)
    for wi, kk in enumerate(("bq", "bk", "bv", "bo")):
        bb_ = np.asarray(inputs[kk], dtype=np.float32)
        for h in range(H):
            bh[:, wi * 4 + h] = bb_[DK * h : DK * h + DK]
    # bv folded into output bias: attention rows sum to 1, so the +bv inside
    # vh passes through to x additively; x@Wo.T + bo == pv@Wo.T + (Wo@bv + bo)
    bo2 = (
        np.asarray(inputs["bo"], dtype=np.float32)
        + np.asarray(inputs["Wo"], dtype=np.float32)
        @ np.asarray(inputs["bv"], dtype=np.float32)
    ).reshape(K, 1)

    common = {
        "sdt": sdt,
        "ident": np.eye(128, dtype=np.float32),
        "identb": np.eye(128, dtype=np.float32).astype(bfd),
        "wst": wst,
        "wl": wlo,
        "bh": bh,
        "bo_col": bo2,
        "ones_row": np.ones((1, V), bfd),
        "ones_col": np.ones((128, V // 128), bfd),
    }

    in_maps = []
    for core in range(8):
        bb, half = core // 2, core % 2
        if half == 0:
            vb, ib = verts[bb], idx[bb]
        else:
            perm = np.concatenate([np.arange(VQ, V), np.arange(0, VQ)])
            vb = verts[bb][perm]
            ib = np.where(idx[bb][perm] >= VQ, idx[bb][perm] - VQ, idx[bb][perm] + VQ)
        in_maps.append({
            "verts": np.ascontiguousarray(vb),
            "gath": np.ascontiguousarray(vb[ib]),
            **common,
        })
    return in_maps


def run(inputs, trace=False, trace_kwargs=None):
    from concourse.bass_utils import run_bass_kernel_spmd

    if "nc" not in _CACHE:
        _CACHE["nc"] = _build_program()
    nc = _CACHE["nc"]
    in_maps = _host_prep(inputs)
    res = run_bass_kernel_spmd(
        nc, in_maps, core_ids=list(range(8)), trace=trace,
        **(trace_kwargs or {}),
    )
    out = np.zeros((BS, V, K), np.float32)
    for core in range(8):
        bb, half = core // 2, core % 2
        ot = res.results[core]["out_t"]  # [64, 1024]
        out[bb, half * VQ : half * VQ + VQ, :] = ot.T
    return out, res


def kernel(**inputs) -> np.ndarray:
    out, _ = run(inputs, trace=False)
    return out
